# revision 10
# baseline (speedup 1.0000x reference)
"""DGI (Deep Graph Infomax) forward kernel for 8 TRN2 NeuronCores.

Problem (all shapes hardcoded):
  seq1, seq2: [1, 8192, 128] f32   node features
  adj:        [1, 8192, 8192] f32  dense adjacency
  cc_label:   [8, 1024] i32        community partition (arange layout)
  W: [128,128], b: [128], Wb: [128,128], bb: [] f32
  out:        [1, 16384] f32       = concat(ret1, ret2)

Math per GCN branch: h = relu(adj @ (seq @ W) + b). The seq @ W product is
tiny and precomputed on the host, so the device program is one big fp16
contraction per branch plus a short epilogue:

  ZW[h, n] = sum_m seqW_s[m, h] * adjT[m, n]   (256 fp16 matmuls into 4 psum
                                                banks, accumulating over all
                                                64 m-tiles)
  h256     = relu(ZW + 256*b)                  (scalar engine for branch 0
                                                chunks with fused community
                                                accumulation; vector
                                                tensor_scalar add+max for
                                                branch 1)
  c        = sigmoid(csum / (1024*256))        [128, 1] f16
  cw       = (Wb^T/256) @ c                    [128, 1] f16 (undoes the 256)
  sc_s[n]  = sum_h h256_s[h, n] * cw[h] + bb   [1, 1024] per branch

adj is pre-scaled by 256 on the host so fp16(adj*256) sits in the normal
range; the scale rides through h256 and is undone in cw, so no separate
rescale op exists anywhere.

Sharding: core k owns nodes [1024k, 1024k+1024) == community k (cc_label
is arange). Each core reads its adjT column block (16.8 MB fp16) and the
precomputed seqW for both branches (4 MB, replicated). No collectives.

DMA schedule: the PE's first matmul needs only seqW tiles 0-3 and adjT
tile 0, so those lead the sync queue (lowest startup latency), followed by
adj groups that grow from 2 to 4 tiles. The scalar queue weaves the
remaining seqW between its share of adj groups. Every transfer lands >=1us
before the PE needs it at the 864 ns/m-tile consumption rate, with the
scalar queue's items given >=2.8us margin against its slower spin-up.
Activation tables for Relu/Sigmoid/Identity are prefetched via dummy [1,1]
activations so no ACT_TABLE_LOAD lands on the epilogue's critical path.
"""

import numpy as np

import concourse.bass as bass
import concourse.tile as tile
from concourse import bacc, mybir
from concourse.bass_utils import run_bass_kernel_spmd

N = 8192          # nodes
D = 128           # input feature dim
H = 128           # hidden dim
NC = 8            # communities / cores
CS = N // NC      # community size (nodes per core)
MT = N // 128     # number of 128-row m-tiles (64)
CHUNK = 512       # matmul moving free dim (psum bank width in fp32)
NCH = CS // CHUNK # n-chunks per core (2)

F32 = mybir.dt.float32
F16 = mybir.dt.float16
ADJ_SCALE = 256.0

# Queue plans: ("a", lo, hi) = adjT m-tiles [lo:hi), ("w", lo, hi) = seqW
# m-tiles. Per-queue order is transfer order.
SYNC_PLAN = [
    ("w", 0, 2), ("a", 0, 1), ("a", 1, 2), ("w", 2, 6), ("a", 2, 4),
    ("a", 4, 6), ("a", 8, 12), ("a", 16, 20), ("a", 24, 28), ("a", 32, 36),
    ("a", 40, 44), ("a", 48, 52), ("a", 56, 60),
]
SCALAR_PLAN = [
    ("w", 6, 14), ("a", 6, 8), ("w", 14, 20), ("a", 12, 16), ("w", 20, 36),
    ("a", 20, 24), ("a", 28, 32), ("w", 36, 52), ("a", 36, 40),
    ("a", 44, 48), ("w", 52, 64), ("a", 52, 56), ("a", 60, 64),
]
_adj = sorted(r for k, *r in SYNC_PLAN + SCALAR_PLAN if k == "a")
_sw = sorted(r for k, *r in SYNC_PLAN + SCALAR_PLAN if k == "w")
for runs in (_adj, _sw):
    assert runs[0][0] == 0 and runs[-1][1] == MT
    assert all(a[1] == b[0] for a, b in zip(runs, runs[1:]))


def _build_module() -> bass.Bass:
    nc = bacc.Bacc()

    adjt = nc.declare_dram_parameter("adjt", [128, MT, CS], F16, isOutput=False)
    seqw = nc.declare_dram_parameter("seqw", [128, MT, 2, H], F16, isOutput=False)
    wbts = nc.declare_dram_parameter("wbts", [H, H], F16, isOutput=False)
    b256 = nc.declare_dram_parameter("b256", [H, 1], F32, isOutput=False)
    bbvec = nc.declare_dram_parameter("bbvec", [1, 1], F32, isOutput=False)
    out = nc.declare_dram_parameter("out", [2, CS], F32, isOutput=True)

    with tile.TileContext(nc) as tc:
        _emit(tc, adjt, seqw, wbts, b256, bbvec, out)
    nc.finalize()
    return nc


def _emit(tc, adjt, seqw, wbts, b256, bbvec, out):
    nc = tc.nc
    AF = mybir.ActivationFunctionType
    with (
        tc.tile_pool(name="singles", bufs=1) as singles,
        tc.tile_pool(name="psum", bufs=1, space="PSUM") as psum,
    ):
        seqw_sb = singles.tile([128, MT, 2, H], F16)
        adj_sb = singles.tile([128, MT, CS], F16)

        def issue(eng, plan):
            for kind, a, b in plan:
                if kind == "w":
                    eng.dma_start(out=seqw_sb[:, a:b, :, :], in_=seqw[:, a:b, :, :])
                else:
                    eng.dma_start(out=adj_sb[:, a:b, :], in_=adjt[:, a:b, :])

        issue(nc.sync, SYNC_PLAN)
        issue(nc.scalar, SCALAR_PLAN)

        wbts_sb = singles.tile([H, H], F16)
        nc.gpsimd.dma_start(out=wbts_sb, in_=wbts[:])
        b256_sb = singles.tile([H, 1], F32)
        nc.gpsimd.dma_start(out=b256_sb, in_=b256[:])
        bb_sb = singles.tile([1, 1], F32)
        nc.gpsimd.dma_start(out=bb_sb, in_=bbvec[:])

        # Prefetch the activation tables during the DMA ramp: dummy [1,1]
        # activations against the first seqW tile (the earliest data).
        scratch = singles.tile([1, 1], F32)
        probe = seqw_sb[0:1, 0, 0, 0:1]
        for fn in (AF.Relu, AF.Sigmoid, AF.Identity):
            nc.scalar.activation(out=scratch, in_=probe, func=fn)

        # zw[s][c]: one psum bank per (branch, n-chunk); [:1, s, c, :] rows
        # are reused for the score matmuls afterwards.
        zw = psum.tile([128, 2, NCH, CHUNK], F32)
        h_sb = [
            [singles.tile([128, CHUNK], F16, name=f"h_sb_{s}_{c}") for c in range(NCH)]
            for s in range(2)
        ]
        csum = [singles.tile([H, 1], F32, name=f"csum_{c}") for c in range(NCH)]

        def mm(t, s):
            lhsT = seqw_sb[:, t, s, :]
            for c in range(NCH):
                nc.tensor.matmul(
                    zw[:, s, c, :],
                    lhsT,
                    adj_sb[:, t, c * CHUNK : (c + 1) * CHUNK],
                    start=(t == 0),
                    stop=(t == MT - 1),
                )

        # Main stream, with branch 0's last HOIST m-tiles hoisted ahead of
        # branch 1's so the whole sigmoid/cw chain overlaps the tail of the
        # matmul stream instead of trailing it.
        HOIST = 10
        for t in range(MT - HOIST):
            for s in range(2):
                mm(t, s)
        for t in range(MT - HOIST, MT):
            mm(t, 0)

        # Branch-0 epilogue: chunk 0 on scalar (fused community accumulate),
        # chunk 1 on vector; these run while the PE works branch 1's tail.
        add, mx = mybir.AluOpType.add, mybir.AluOpType.max
        nc.scalar.activation(
            out=h_sb[0][0],
            in_=zw[:, 0, 0, :],
            func=AF.Relu,
            bias=b256_sb,
            accum_out=csum[0],
        )
        nc.vector.tensor_scalar(
            out=h_sb[0][1],
            in0=zw[:, 0, 1, :],
            scalar1=b256_sb,
            scalar2=0.0,
            op0=add,
            op1=mx,
        )
        nc.vector.tensor_reduce(
            out=csum[1],
            in_=h_sb[0][1],
            axis=mybir.AxisListType.X,
            op=add,
        )
        csum_tot = singles.tile([H, 1], F32)
        nc.vector.tensor_add(out=csum_tot, in0=csum[0], in1=csum[1])
        c_sb = singles.tile([H, 1], F16)
        nc.scalar.activation(
            out=c_sb, in_=csum_tot, func=AF.Sigmoid, scale=1.0 / (CS * ADJ_SCALE)
        )

        for t in range(MT - HOIST, MT - 6):
            mm(t, 1)
        # cw matmul slotted inside branch 1's tail; the sigmoid result has
        # comfortable margin by this issue slot, so the PE never blocks.
        cw_ps = psum.tile([H, 1], F32)
        nc.tensor.matmul(cw_ps, wbts_sb, c_sb, start=True, stop=True)
        for t in range(MT - 6, MT):
            mm(t, 1)

        cw_sb = singles.tile([H, 1], F16)
        nc.vector.tensor_copy(out=cw_sb, in_=cw_ps)

        nc.scalar.activation(
            out=h_sb[1][0], in_=zw[:, 1, 0, :], func=AF.Relu, bias=b256_sb
        )
        nc.vector.tensor_scalar(
            out=h_sb[1][1],
            in0=zw[:, 1, 1, :],
            scalar1=b256_sb,
            scalar2=0.0,
            op0=add,
            op1=mx,
        )

        out_sb = singles.tile([1, 2, NCH, CHUNK], F32)
        for s in range(2):
            for c in range(NCH):
                nc.tensor.matmul(
                    zw[:1, s, c, :], cw_sb, h_sb[s][c], start=True, stop=True
                )
            # bb add + psum->sbuf, one 512-chunk on vector and one on
            # scalar so each branch's pair runs concurrently.
            for c in range(NCH):
                dst = out_sb[:, s, c, :]
                src_ps = zw[:1, s, c, :]
                if c == 0:
                    nc.vector.tensor_scalar_add(out=dst, in0=src_ps, scalar1=bb_sb)
                else:
                    nc.scalar.activation(
                        out=dst, in_=src_ps, func=AF.Identity, bias=bb_sb
                    )
            nc.sync.dma_start(
                out=out[s : s + 1, :].unsqueeze(0),
                in_=out_sb[:, s, :, :].unsqueeze(1),
            )


_MODULE_CACHE: list = []


def get_module() -> bass.Bass:
    if not _MODULE_CACHE:
        _MODULE_CACHE.append(_build_module())
    return _MODULE_CACHE[0]


def shard_inputs(inputs: dict) -> list[dict]:
    """Full inputs -> per-core input maps (row-block sharding of adjT)."""
    w = np.asarray(inputs["W"], np.float32)
    sw = [
        (np.asarray(inputs[k], np.float32)[0] @ w).astype(np.float16)
        for k in ("seq1", "seq2")
    ]  # each [N, H]
    # seqw[p, t, s, h] = seqW_s[128*t + p, h]
    seqw = np.ascontiguousarray(
        np.stack(sw, axis=0).reshape(2, MT, 128, H).transpose(2, 1, 0, 3)
    )
    adj16 = (np.asarray(inputs["adj"], np.float32)[0] * ADJ_SCALE).astype(np.float16)
    wbts = np.ascontiguousarray(
        (np.asarray(inputs["Wb"], np.float32).T / ADJ_SCALE).astype(np.float16)
    )
    b256 = (np.asarray(inputs["b"], np.float32) * ADJ_SCALE).reshape(H, 1).copy()
    bbvec = np.asarray(inputs["bb"], np.float32).reshape(1, 1).copy()

    in_maps = []
    for k in range(NC):
        in_maps.append(
            {
                "adjt": np.ascontiguousarray(
                    adj16[k * CS : (k + 1) * CS, :].T.reshape(MT, 128, CS).transpose(1, 0, 2)
                ),
                "seqw": seqw,
                "wbts": wbts,
                "b256": b256,
                "bbvec": bbvec,
            }
        )
    return in_maps


def gather_output(core_outs: list[np.ndarray], cc_label: np.ndarray) -> np.ndarray:
    """Per-core [2, CS] score blocks -> full [1, 2N] output.

    Scatter through cc_label mirrors the reference's .at[flat].set: entry
    (community k, position j) is the score of node cc_label[k, j].
    """
    sc1 = np.concatenate([o[0] for o in core_outs]).astype(np.float32)
    sc2 = np.concatenate([o[1] for o in core_outs]).astype(np.float32)
    flat = np.asarray(cc_label).reshape(-1)
    ret1 = np.zeros(N, np.float32)
    ret2 = np.zeros(N, np.float32)
    ret1[flat] = sc1
    ret2[flat] = sc2
    return np.concatenate([ret1, ret2])[None, :]


def kernel(**inputs) -> np.ndarray:
    nc = get_module()
    in_maps = shard_inputs(inputs)
    res = run_bass_kernel_spmd(nc, in_maps, core_ids=list(range(NC)))
    core_outs = [res.results[k]["out"] for k in range(NC)]
    return gather_output(core_outs, inputs["cc_label"])


if __name__ == "__main__":
    nc = get_module()
    print("module built ok")


# revision 11
# speedup vs baseline: 1.0580x; 1.0580x over previous
"""DGI (Deep Graph Infomax) forward kernel for 8 TRN2 NeuronCores.

Problem (all shapes hardcoded):
  seq1, seq2: [1, 8192, 128] f32   node features
  adj:        [1, 8192, 8192] f32  dense adjacency
  cc_label:   [8, 1024] i32        community partition (arange layout)
  W: [128,128], b: [128], Wb: [128,128], bb: [] f32
  out:        [1, 16384] f32       = concat(ret1, ret2)

Math per GCN branch: h = relu(adj @ (seq @ W) + b). The seq @ W product is
tiny and precomputed on the host, so the device program is one big fp16
contraction per branch plus a short epilogue:

  ZW[h, n] = sum_m seqW_s[m, h] * adjT[m, n]   (256 fp16 matmuls into 4 psum
                                                banks, accumulating over all
                                                64 m-tiles)
  h256     = relu(ZW + 256*b)                  (scalar engine for branch 0
                                                chunks with fused community
                                                accumulation; vector
                                                tensor_scalar add+max for
                                                branch 1)
  c        = sigmoid(csum / (1024*256))        [128, 1] f16
  cw       = (Wb^T/256) @ c                    [128, 1] f16 (undoes the 256)
  sc_s[n]  = sum_h h256_s[h, n] * cw[h] + bb   [1, 1024] per branch

adj is pre-scaled by 256 on the host so fp16(adj*256) sits in the normal
range; the scale rides through h256 and is undone in cw, so no separate
rescale op exists anywhere.

Sharding: core k owns nodes [1024k, 1024k+1024) == community k (cc_label
is arange). Each core reads its adjT column block (16.8 MB fp16) and the
precomputed seqW for both branches (4 MB, replicated). No collectives.

DMA schedule: the PE's first matmul needs only seqW tiles 0-3 and adjT
tile 0, so those lead the sync queue (lowest startup latency), followed by
adj groups that grow from 2 to 4 tiles. The scalar queue weaves the
remaining seqW between its share of adj groups. Every transfer lands >=1us
before the PE needs it at the 864 ns/m-tile consumption rate, with the
scalar queue's items given >=2.8us margin against its slower spin-up.
Activation tables for Relu/Sigmoid/Identity are prefetched via dummy [1,1]
activations so no ACT_TABLE_LOAD lands on the epilogue's critical path.
"""

import numpy as np

import concourse.bass as bass
import concourse.tile as tile
from concourse import bacc, mybir
from concourse.bass_utils import run_bass_kernel_spmd

N = 8192          # nodes
D = 128           # input feature dim
H = 128           # hidden dim
NC = 8            # communities / cores
CS = N // NC      # community size (nodes per core)
MT = N // 128     # number of 128-row m-tiles (64)
CHUNK = 512       # matmul moving free dim (psum bank width in fp32)
NCH = CS // CHUNK # n-chunks per core (2)

F32 = mybir.dt.float32
F16 = mybir.dt.float16
ADJ_SCALE = 256.0

# Queue plans: ("a", lo, hi) = adjT m-tiles [lo:hi), ("w", lo, hi) = seqW
# m-tiles. Per-queue order is transfer order.
SYNC_PLAN = [
    ("w", 0, 4), ("a", 0, 2), ("a", 2, 4), ("a", 4, 6), ("a", 8, 12),
    ("a", 16, 20), ("a", 24, 28), ("a", 32, 36), ("a", 40, 44),
    ("a", 48, 52), ("a", 56, 60),
]
SCALAR_PLAN = [
    ("w", 4, 12), ("a", 6, 8), ("w", 12, 20), ("a", 12, 16), ("w", 20, 36),
    ("a", 20, 24), ("a", 28, 32), ("w", 36, 52), ("a", 36, 40),
    ("a", 44, 48), ("w", 52, 64), ("a", 52, 56), ("a", 60, 64),
]
_adj = sorted(r for k, *r in SYNC_PLAN + SCALAR_PLAN if k == "a")
_sw = sorted(r for k, *r in SYNC_PLAN + SCALAR_PLAN if k == "w")
for runs in (_adj, _sw):
    assert runs[0][0] == 0 and runs[-1][1] == MT
    assert all(a[1] == b[0] for a, b in zip(runs, runs[1:]))


def _build_module() -> bass.Bass:
    nc = bacc.Bacc()

    adjt = nc.declare_dram_parameter("adjt", [128, MT, CS], F16, isOutput=False)
    seqw = nc.declare_dram_parameter("seqw", [128, MT, 2, H], F16, isOutput=False)
    wbts = nc.declare_dram_parameter("wbts", [H, H], F16, isOutput=False)
    b256 = nc.declare_dram_parameter("b256", [H, 1], F32, isOutput=False)
    bbvec = nc.declare_dram_parameter("bbvec", [1, 1], F32, isOutput=False)
    out = nc.declare_dram_parameter("out", [2, CS], F32, isOutput=True)

    with tile.TileContext(nc) as tc:
        _emit(tc, adjt, seqw, wbts, b256, bbvec, out)
    nc.finalize()
    return nc


def _emit(tc, adjt, seqw, wbts, b256, bbvec, out):
    nc = tc.nc
    AF = mybir.ActivationFunctionType
    with (
        tc.tile_pool(name="singles", bufs=1) as singles,
        tc.tile_pool(name="psum", bufs=1, space="PSUM") as psum,
    ):
        seqw_sb = singles.tile([128, MT, 2, H], F16)
        adj_sb = singles.tile([128, MT, CS], F16)

        def issue(eng, plan):
            for kind, a, b in plan:
                if kind == "w":
                    eng.dma_start(out=seqw_sb[:, a:b, :, :], in_=seqw[:, a:b, :, :])
                else:
                    eng.dma_start(out=adj_sb[:, a:b, :], in_=adjt[:, a:b, :])

        issue(nc.sync, SYNC_PLAN)
        issue(nc.scalar, SCALAR_PLAN)

        wbts_sb = singles.tile([H, H], F16)
        nc.gpsimd.dma_start(out=wbts_sb, in_=wbts[:])
        b256_sb = singles.tile([H, 1], F32)
        nc.gpsimd.dma_start(out=b256_sb, in_=b256[:])
        bb_sb = singles.tile([1, 1], F32)
        nc.gpsimd.dma_start(out=bb_sb, in_=bbvec[:])

        # Prefetch the activation tables during the DMA ramp: dummy [1,1]
        # activations against the first seqW tile (the earliest data).
        scratch = singles.tile([1, 1], F32)
        probe = seqw_sb[0:1, 0, 0, 0:1]
        for fn in (AF.Relu, AF.Sigmoid, AF.Identity):
            nc.scalar.activation(out=scratch, in_=probe, func=fn)

        # zw[s][c]: one psum bank per (branch, n-chunk), as four separate
        # tiles so branch 1's tail matmuls carry no false dependency on
        # branch 0's relu reads; [:1, :] rows are reused for the score
        # matmuls afterwards.
        zw = [
            [psum.tile([128, CHUNK], F32, name=f"zw_{s}_{c}") for c in range(NCH)]
            for s in range(2)
        ]
        h_sb = [
            [singles.tile([128, CHUNK], F16, name=f"h_sb_{s}_{c}") for c in range(NCH)]
            for s in range(2)
        ]
        csum = [singles.tile([H, 1], F32, name=f"csum_{c}") for c in range(NCH)]

        def mm(t, s):
            lhsT = seqw_sb[:, t, s, :]
            for c in range(NCH):
                nc.tensor.matmul(
                    zw[s][c],
                    lhsT,
                    adj_sb[:, t, c * CHUNK : (c + 1) * CHUNK],
                    start=(t == 0),
                    stop=(t == MT - 1),
                )

        # Main stream, with branch 0's last HOIST m-tiles hoisted ahead of
        # branch 1's so the whole sigmoid/cw chain overlaps the tail of the
        # matmul stream instead of trailing it.
        HOIST = 10
        for t in range(MT - HOIST):
            for s in range(2):
                mm(t, s)
        for t in range(MT - HOIST, MT):
            mm(t, 0)

        # Branch-0 epilogue: chunk 0 on scalar (fused community accumulate),
        # chunk 1 on vector; these run while the PE works branch 1's tail.
        add, mx = mybir.AluOpType.add, mybir.AluOpType.max
        nc.scalar.activation(
            out=h_sb[0][0],
            in_=zw[0][0],
            func=AF.Relu,
            bias=b256_sb,
            accum_out=csum[0],
        )
        nc.vector.tensor_scalar(
            out=h_sb[0][1],
            in0=zw[0][1],
            scalar1=b256_sb,
            scalar2=0.0,
            op0=add,
            op1=mx,
        )
        nc.vector.tensor_reduce(
            out=csum[1],
            in_=h_sb[0][1],
            axis=mybir.AxisListType.X,
            op=add,
        )
        csum_tot = singles.tile([H, 1], F32)
        nc.vector.tensor_add(out=csum_tot, in0=csum[0], in1=csum[1])
        c_sb = singles.tile([H, 1], F16)
        nc.scalar.activation(
            out=c_sb, in_=csum_tot, func=AF.Sigmoid, scale=1.0 / (CS * ADJ_SCALE)
        )

        for t in range(MT - HOIST, MT - 6):
            mm(t, 1)
        # cw matmul slotted inside branch 1's tail; the sigmoid result has
        # comfortable margin by this issue slot, so the PE never blocks.
        cw_ps = psum.tile([H, 1], F32)
        nc.tensor.matmul(cw_ps, wbts_sb, c_sb, start=True, stop=True)
        for t in range(MT - 6, MT):
            mm(t, 1)

        cw_sb = singles.tile([H, 1], F16)
        nc.vector.tensor_copy(out=cw_sb, in_=cw_ps)

        nc.scalar.activation(
            out=h_sb[1][0], in_=zw[1][0], func=AF.Relu, bias=b256_sb
        )
        nc.vector.tensor_scalar(
            out=h_sb[1][1],
            in0=zw[1][1],
            scalar1=b256_sb,
            scalar2=0.0,
            op0=add,
            op1=mx,
        )

        out_sb = singles.tile([1, 2, NCH, CHUNK], F32)
        for s in range(2):
            for c in range(NCH):
                nc.tensor.matmul(
                    zw[s][c][:1, :], cw_sb, h_sb[s][c], start=True, stop=True
                )
            # bb add + psum->sbuf, one 512-chunk on vector and one on
            # scalar so each branch's pair runs concurrently.
            for c in range(NCH):
                dst = out_sb[:, s, c, :]
                src_ps = zw[s][c][:1, :]
                if c == 0:
                    nc.vector.tensor_scalar_add(out=dst, in0=src_ps, scalar1=bb_sb)
                else:
                    nc.scalar.activation(
                        out=dst, in_=src_ps, func=AF.Identity, bias=bb_sb
                    )
            nc.sync.dma_start(
                out=out[s : s + 1, :].unsqueeze(0),
                in_=out_sb[:, s, :, :].unsqueeze(1),
            )


_MODULE_CACHE: list = []


def get_module() -> bass.Bass:
    if not _MODULE_CACHE:
        _MODULE_CACHE.append(_build_module())
    return _MODULE_CACHE[0]


def shard_inputs(inputs: dict) -> list[dict]:
    """Full inputs -> per-core input maps (row-block sharding of adjT)."""
    w = np.asarray(inputs["W"], np.float32)
    sw = [
        (np.asarray(inputs[k], np.float32)[0] @ w).astype(np.float16)
        for k in ("seq1", "seq2")
    ]  # each [N, H]
    # seqw[p, t, s, h] = seqW_s[128*t + p, h]
    seqw = np.ascontiguousarray(
        np.stack(sw, axis=0).reshape(2, MT, 128, H).transpose(2, 1, 0, 3)
    )
    adj16 = (np.asarray(inputs["adj"], np.float32)[0] * ADJ_SCALE).astype(np.float16)
    wbts = np.ascontiguousarray(
        (np.asarray(inputs["Wb"], np.float32).T / ADJ_SCALE).astype(np.float16)
    )
    b256 = (np.asarray(inputs["b"], np.float32) * ADJ_SCALE).reshape(H, 1).copy()
    bbvec = np.asarray(inputs["bb"], np.float32).reshape(1, 1).copy()

    in_maps = []
    for k in range(NC):
        in_maps.append(
            {
                "adjt": np.ascontiguousarray(
                    adj16[k * CS : (k + 1) * CS, :].T.reshape(MT, 128, CS).transpose(1, 0, 2)
                ),
                "seqw": seqw,
                "wbts": wbts,
                "b256": b256,
                "bbvec": bbvec,
            }
        )
    return in_maps


def gather_output(core_outs: list[np.ndarray], cc_label: np.ndarray) -> np.ndarray:
    """Per-core [2, CS] score blocks -> full [1, 2N] output.

    Scatter through cc_label mirrors the reference's .at[flat].set: entry
    (community k, position j) is the score of node cc_label[k, j].
    """
    sc1 = np.concatenate([o[0] for o in core_outs]).astype(np.float32)
    sc2 = np.concatenate([o[1] for o in core_outs]).astype(np.float32)
    flat = np.asarray(cc_label).reshape(-1)
    ret1 = np.zeros(N, np.float32)
    ret2 = np.zeros(N, np.float32)
    ret1[flat] = sc1
    ret2[flat] = sc2
    return np.concatenate([ret1, ret2])[None, :]


def kernel(**inputs) -> np.ndarray:
    nc = get_module()
    in_maps = shard_inputs(inputs)
    res = run_bass_kernel_spmd(nc, in_maps, core_ids=list(range(NC)))
    core_outs = [res.results[k]["out"] for k in range(NC)]
    return gather_output(core_outs, inputs["cc_label"])


if __name__ == "__main__":
    nc = get_module()
    print("module built ok")


# revision 12
# speedup vs baseline: 1.0591x; 1.0011x over previous
"""DGI (Deep Graph Infomax) forward kernel for 8 TRN2 NeuronCores.

Problem (all shapes hardcoded):
  seq1, seq2: [1, 8192, 128] f32   node features
  adj:        [1, 8192, 8192] f32  dense adjacency
  cc_label:   [8, 1024] i32        community partition (arange layout)
  W: [128,128], b: [128], Wb: [128,128], bb: [] f32
  out:        [1, 16384] f32       = concat(ret1, ret2)

Math per GCN branch: h = relu(adj @ (seq @ W) + b). The seq @ W product is
tiny and precomputed on the host, so the device program is one big fp16
contraction per branch plus a short epilogue:

  ZW[h, n] = sum_m seqW_s[m, h] * adjT[m, n]   (256 fp16 matmuls into 4 psum
                                                banks, accumulating over all
                                                64 m-tiles)
  h256     = relu(ZW + 256*b)                  (scalar engine for branch 0
                                                chunks with fused community
                                                accumulation; vector
                                                tensor_scalar add+max for
                                                branch 1)
  c        = sigmoid(csum / (1024*256))        [128, 1] f16
  cw       = (Wb^T/256) @ c                    [128, 1] f16 (undoes the 256)
  sc_s[n]  = sum_h h256_s[h, n] * cw[h] + bb   [1, 1024] per branch

adj is pre-scaled by 256 on the host so fp16(adj*256) sits in the normal
range; the scale rides through h256 and is undone in cw, so no separate
rescale op exists anywhere.

Sharding: core k owns nodes [1024k, 1024k+1024) == community k (cc_label
is arange). Each core reads its adjT column block (16.8 MB fp16) and the
precomputed seqW for both branches (4 MB, replicated). No collectives.

DMA schedule: the PE's first matmul needs only seqW tiles 0-3 and adjT
tile 0, so those lead the sync queue (lowest startup latency), followed by
adj groups that grow from 2 to 4 tiles. The scalar queue weaves the
remaining seqW between its share of adj groups. Every transfer lands >=1us
before the PE needs it at the 864 ns/m-tile consumption rate, with the
scalar queue's items given >=2.8us margin against its slower spin-up.
Activation tables for Relu/Sigmoid/Identity are prefetched via dummy [1,1]
activations so no ACT_TABLE_LOAD lands on the epilogue's critical path.
"""

import numpy as np

import concourse.bass as bass
import concourse.tile as tile
from concourse import bacc, mybir
from concourse.bass_utils import run_bass_kernel_spmd

N = 8192          # nodes
D = 128           # input feature dim
H = 128           # hidden dim
NC = 8            # communities / cores
CS = N // NC      # community size (nodes per core)
MT = N // 128     # number of 128-row m-tiles (64)
CHUNK = 512       # matmul moving free dim (psum bank width in fp32)
NCH = CS // CHUNK # n-chunks per core (2)

F32 = mybir.dt.float32
F16 = mybir.dt.float16
ADJ_SCALE = 256.0

# Queue plans: ("a", lo, hi) = adjT m-tiles [lo:hi), ("w", lo, hi) = seqW
# m-tiles. Per-queue order is transfer order.
SYNC_PLAN = [
    ("w", 0, 4), ("a", 0, 2), ("a", 2, 4), ("a", 4, 6), ("a", 8, 12),
    ("a", 16, 20), ("a", 24, 28), ("a", 32, 36), ("a", 36, 40),
    ("a", 40, 44), ("a", 48, 52), ("a", 56, 60),
]
SCALAR_PLAN = [
    ("w", 4, 12), ("a", 6, 8), ("w", 12, 20), ("a", 12, 16), ("w", 20, 36),
    ("a", 20, 24), ("a", 28, 32), ("w", 36, 52), ("w", 52, 58),
    ("a", 44, 48), ("a", 52, 56), ("w", 58, 64), ("a", 60, 64),
]
_adj = sorted(r for k, *r in SYNC_PLAN + SCALAR_PLAN if k == "a")
_sw = sorted(r for k, *r in SYNC_PLAN + SCALAR_PLAN if k == "w")
for runs in (_adj, _sw):
    assert runs[0][0] == 0 and runs[-1][1] == MT
    assert all(a[1] == b[0] for a, b in zip(runs, runs[1:]))


def _build_module() -> bass.Bass:
    nc = bacc.Bacc()

    adjt = nc.declare_dram_parameter("adjt", [128, MT, CS], F16, isOutput=False)
    seqw = nc.declare_dram_parameter("seqw", [128, MT, 2, H], F16, isOutput=False)
    wbts = nc.declare_dram_parameter("wbts", [H, H], F16, isOutput=False)
    b256 = nc.declare_dram_parameter("b256", [H, 1], F32, isOutput=False)
    bbvec = nc.declare_dram_parameter("bbvec", [1, 1], F32, isOutput=False)
    out = nc.declare_dram_parameter("out", [2, CS], F32, isOutput=True)

    with tile.TileContext(nc) as tc:
        _emit(tc, adjt, seqw, wbts, b256, bbvec, out)
    nc.finalize()
    return nc


def _emit(tc, adjt, seqw, wbts, b256, bbvec, out):
    nc = tc.nc
    AF = mybir.ActivationFunctionType
    with (
        tc.tile_pool(name="singles", bufs=1) as singles,
        tc.tile_pool(name="psum", bufs=1, space="PSUM") as psum,
    ):
        seqw_sb = singles.tile([128, MT, 2, H], F16)
        adj_sb = singles.tile([128, MT, CS], F16)

        def issue(eng, plan):
            for kind, a, b in plan:
                if kind == "w":
                    eng.dma_start(out=seqw_sb[:, a:b, :, :], in_=seqw[:, a:b, :, :])
                else:
                    eng.dma_start(out=adj_sb[:, a:b, :], in_=adjt[:, a:b, :])

        issue(nc.sync, SYNC_PLAN)
        issue(nc.scalar, SCALAR_PLAN)

        wbts_sb = singles.tile([H, H], F16)
        nc.gpsimd.dma_start(out=wbts_sb, in_=wbts[:])
        b256_sb = singles.tile([H, 1], F32)
        nc.gpsimd.dma_start(out=b256_sb, in_=b256[:])
        bb_sb = singles.tile([1, 1], F32)
        nc.gpsimd.dma_start(out=bb_sb, in_=bbvec[:])

        # Prefetch the activation tables during the DMA ramp: dummy [1,1]
        # activations against the first seqW tile (the earliest data).
        scratch = singles.tile([1, 1], F32)
        probe = seqw_sb[0:1, 0, 0, 0:1]
        for fn in (AF.Relu, AF.Sigmoid, AF.Identity):
            nc.scalar.activation(out=scratch, in_=probe, func=fn)

        # zw[s][c]: one psum bank per (branch, n-chunk), as four separate
        # tiles so branch 1's tail matmuls carry no false dependency on
        # branch 0's relu reads; [:1, :] rows are reused for the score
        # matmuls afterwards.
        zw = [
            [psum.tile([128, CHUNK], F32, name=f"zw_{s}_{c}") for c in range(NCH)]
            for s in range(2)
        ]
        h_sb = [
            [singles.tile([128, CHUNK], F16, name=f"h_sb_{s}_{c}") for c in range(NCH)]
            for s in range(2)
        ]
        csum = [singles.tile([H, 1], F32, name=f"csum_{c}") for c in range(NCH)]

        def mm(t, s):
            lhsT = seqw_sb[:, t, s, :]
            for c in range(NCH):
                nc.tensor.matmul(
                    zw[s][c],
                    lhsT,
                    adj_sb[:, t, c * CHUNK : (c + 1) * CHUNK],
                    start=(t == 0),
                    stop=(t == MT - 1),
                )

        # Main stream, with branch 0's last HOIST m-tiles hoisted ahead of
        # branch 1's so the whole sigmoid/cw chain overlaps the tail of the
        # matmul stream instead of trailing it.
        HOIST = 14
        for t in range(MT - HOIST):
            for s in range(2):
                mm(t, s)
        for t in range(MT - HOIST, MT):
            mm(t, 0)

        # Branch-0 epilogue: chunk 0 on scalar (fused community accumulate),
        # chunk 1 on vector; these run while the PE works branch 1's tail.
        add, mx = mybir.AluOpType.add, mybir.AluOpType.max
        nc.scalar.activation(
            out=h_sb[0][0],
            in_=zw[0][0],
            func=AF.Relu,
            bias=b256_sb,
            accum_out=csum[0],
        )
        nc.vector.tensor_scalar(
            out=h_sb[0][1],
            in0=zw[0][1],
            scalar1=b256_sb,
            scalar2=0.0,
            op0=add,
            op1=mx,
        )
        nc.vector.tensor_reduce(
            out=csum[1],
            in_=h_sb[0][1],
            axis=mybir.AxisListType.X,
            op=add,
        )
        csum_tot = singles.tile([H, 1], F32)
        nc.vector.tensor_add(out=csum_tot, in0=csum[0], in1=csum[1])
        c_sb = singles.tile([H, 1], F16)
        nc.scalar.activation(
            out=c_sb, in_=csum_tot, func=AF.Sigmoid, scale=1.0 / (CS * ADJ_SCALE)
        )

        for t in range(MT - HOIST, MT):
            mm(t, 1)
        # cw matmul after the stream: sigmoid finished during branch 1's
        # tail, so this never blocks the tensor queue.
        cw_ps = psum.tile([H, 1], F32)
        nc.tensor.matmul(cw_ps, wbts_sb, c_sb, start=True, stop=True)
        cw_sb = singles.tile([H, 1], F16)
        nc.vector.tensor_copy(out=cw_sb, in_=cw_ps)

        nc.scalar.activation(
            out=h_sb[1][0], in_=zw[1][0], func=AF.Relu, bias=b256_sb
        )
        nc.vector.tensor_scalar(
            out=h_sb[1][1],
            in0=zw[1][1],
            scalar1=b256_sb,
            scalar2=0.0,
            op0=add,
            op1=mx,
        )

        out_sb = singles.tile([1, 2, NCH, CHUNK], F32)
        for s in range(2):
            for c in range(NCH):
                nc.tensor.matmul(
                    zw[s][c][:1, :], cw_sb, h_sb[s][c], start=True, stop=True
                )
            # bb add + psum->sbuf, one 512-chunk on vector and one on
            # scalar so each branch's pair runs concurrently.
            for c in range(NCH):
                dst = out_sb[:, s, c, :]
                src_ps = zw[s][c][:1, :]
                if c == 0:
                    nc.vector.tensor_scalar_add(out=dst, in0=src_ps, scalar1=bb_sb)
                else:
                    nc.scalar.activation(
                        out=dst, in_=src_ps, func=AF.Identity, bias=bb_sb
                    )
            nc.sync.dma_start(
                out=out[s : s + 1, :].unsqueeze(0),
                in_=out_sb[:, s, :, :].unsqueeze(1),
            )


_MODULE_CACHE: list = []


def get_module() -> bass.Bass:
    if not _MODULE_CACHE:
        _MODULE_CACHE.append(_build_module())
    return _MODULE_CACHE[0]


def shard_inputs(inputs: dict) -> list[dict]:
    """Full inputs -> per-core input maps (row-block sharding of adjT)."""
    w = np.asarray(inputs["W"], np.float32)
    sw = [
        (np.asarray(inputs[k], np.float32)[0] @ w).astype(np.float16)
        for k in ("seq1", "seq2")
    ]  # each [N, H]
    # seqw[p, t, s, h] = seqW_s[128*t + p, h]
    seqw = np.ascontiguousarray(
        np.stack(sw, axis=0).reshape(2, MT, 128, H).transpose(2, 1, 0, 3)
    )
    adj16 = (np.asarray(inputs["adj"], np.float32)[0] * ADJ_SCALE).astype(np.float16)
    wbts = np.ascontiguousarray(
        (np.asarray(inputs["Wb"], np.float32).T / ADJ_SCALE).astype(np.float16)
    )
    b256 = (np.asarray(inputs["b"], np.float32) * ADJ_SCALE).reshape(H, 1).copy()
    bbvec = np.asarray(inputs["bb"], np.float32).reshape(1, 1).copy()

    in_maps = []
    for k in range(NC):
        in_maps.append(
            {
                "adjt": np.ascontiguousarray(
                    adj16[k * CS : (k + 1) * CS, :].T.reshape(MT, 128, CS).transpose(1, 0, 2)
                ),
                "seqw": seqw,
                "wbts": wbts,
                "b256": b256,
                "bbvec": bbvec,
            }
        )
    return in_maps


def gather_output(core_outs: list[np.ndarray], cc_label: np.ndarray) -> np.ndarray:
    """Per-core [2, CS] score blocks -> full [1, 2N] output.

    Scatter through cc_label mirrors the reference's .at[flat].set: entry
    (community k, position j) is the score of node cc_label[k, j].
    """
    sc1 = np.concatenate([o[0] for o in core_outs]).astype(np.float32)
    sc2 = np.concatenate([o[1] for o in core_outs]).astype(np.float32)
    flat = np.asarray(cc_label).reshape(-1)
    ret1 = np.zeros(N, np.float32)
    ret2 = np.zeros(N, np.float32)
    ret1[flat] = sc1
    ret2[flat] = sc2
    return np.concatenate([ret1, ret2])[None, :]


def kernel(**inputs) -> np.ndarray:
    nc = get_module()
    in_maps = shard_inputs(inputs)
    res = run_bass_kernel_spmd(nc, in_maps, core_ids=list(range(NC)))
    core_outs = [res.results[k]["out"] for k in range(NC)]
    return gather_output(core_outs, inputs["cc_label"])


if __name__ == "__main__":
    nc = get_module()
    print("module built ok")
